# revision 1
# baseline (speedup 1.0000x reference)
"""Residual VQ (Mimi) kernel for 8x TRN2 NeuronCores.

Data-parallel over time: each core processes T/8 = 4096 timesteps.

Per-core algorithm (matches jax fp32 reference bit-closely):
  r_T = (x @ w_in.T).T          kept transposed [256, 4096] as 2x32 [128,128] tiles
  for q in 8 codebooks:
    psum    = 2*r.e_k - etilde_k          (PE: fp32 cross + bf16 aug row)
    s1      = psum - x_sq                 (ACT Identity with per-partition bias)
            = -(x_sq - 2*r.e + etilde) = -dist
    argmin  = max8 + max_index over s1    (DVE; first-index tie-break == jnp.argmin)
    quant   = emb[idx] gather             (indirect DMA)
    r -= quant; out += quant              (PE transpose + DVE, transposed layout)
    x_sq_next = dist_min = -max(s1)       (bias for next layer = max(s1) directly)
  y = out_T.T @ w_out.T

etilde = e_sq rounded to the 2^-17 grid; since fl(x_sq - 2c) lands on that grid
(x_sq in [64,128)), adding etilde is exact and commutes into the PE accumulation,
reproducing the reference's fl(fl(x_sq - 2c) + e_sq) rounding (validated: 1
argmin flip in 262144 vs fp32 reference).
"""
import numpy as np

import concourse.bacc as bacc
import concourse.bass as bass
import concourse.mybir as mybir
import concourse.tile as tile
from concourse.bass_utils import run_bass_kernel_spmd
from concourse.masks import make_identity

F32 = mybir.dt.float32
BF16 = mybir.dt.bfloat16
U32 = mybir.dt.uint32

T, D_IN, D_CB, K, Q = 32768, 512, 256, 2048, 8
import os
NO_GATHER = os.environ.get("VQ_NO_GATHER", "0") == "1"
N_CORES = 8
T_LOC = T // N_CORES          # 4096
NT = T_LOC // 128             # 32 t-tiles
P = 128

Act = mybir.ActivationFunctionType
Alu = mybir.AluOpType


def _build(reps=1):
    nc = bacc.Bacc(None, target_bir_lowering=False, num_swdge_queues=4)

    x = nc.declare_dram_parameter("x", [T_LOC, D_IN], F32, isOutput=False)
    w_in = nc.declare_dram_parameter("w_in", [D_CB, D_IN], F32, isOutput=False)
    w_out = nc.declare_dram_parameter("w_out", [D_IN, D_CB], F32, isOutput=False)
    emb = nc.declare_dram_parameter("emb", [Q * K, D_CB], F32, isOutput=False)
    y = nc.declare_dram_parameter("y", [T_LOC, D_IN], F32, isOutput=True)

    with tile.TileContext(nc) as tc:
      for rep in range(reps):
        R = f"r{rep}_"
        with (
            tc.tile_pool(name=R+"const", bufs=1) as constp,
            tc.tile_pool(name=R+"state", bufs=1) as state,
            tc.tile_pool(name=R+"layer", bufs=2) as layer,
            tc.tile_pool(name=R+"lscratch", bufs=1) as lscratch,
            tc.tile_pool(name=R+"work", bufs=2) as work,
            tc.tile_pool(name=R+"smalls", bufs=4) as smalls,
            tc.tile_pool(name=R+"pdist", bufs=4, space="PSUM") as pdist,
            tc.tile_pool(name=R+"pqt", bufs=2, space="PSUM") as pqt,
            tc.tile_pool(name=R+"paux", bufs=2, space="PSUM") as paux,
        ):
            ident = constp.tile([P, P], F32, tag="ident")
            make_identity(nc, ident[:])
            ones1 = constp.tile([1, P], BF16, tag="ones1")
            nc.gpsimd.memset(ones1[:], 1.0)
            bias_magic = constp.tile([P, 1], F32, tag="bias_magic")
            nc.gpsimd.memset(bias_magic[:], float(2.0 ** 23))
            bias_64 = constp.tile([P, 1], F32, tag="bias_64")
            nc.gpsimd.memset(bias_64[:], 64.0)
            bias_128 = constp.tile([P, 1], F32, tag="bias_128")
            nc.gpsimd.memset(bias_128[:], 128.0)

            w_in_T = constp.tile([P, 4, D_CB], F32, tag="w_in_T")   # [din_p, din_c, dcb]
            w_out_T = constp.tile([P, 2, D_IN], F32, tag="w_out_T")  # [dcb_p, dcb_c, n]

            # r_T, out_T: transposed state, per (dcb-chunk m, t-tile)
            rT = [[state.tile([P, P], F32, tag=f"rT{m}_{t}", name=R+f"rT{m}_{t}")
                   for t in range(NT)] for m in range(2)]
            outT = [[state.tile([P, P], F32, tag=f"oT{m}_{t}", name=R+f"oT{m}_{t}")
                     for t in range(NT)] for m in range(2)]
            # negative x_sq bias, ping-pong across layers
            nxsq = [[state.tile([P, 1], F32, tag=f"nx{s}_{t}", name=R+f"nx{s}_{t}")
                     for t in range(NT)] for s in range(2)]
            augw = [state.tile([2, P], BF16, tag=f"augw_{t}", name=R+f"augw_{t}")
                    for t in range(NT)]
            for t in range(NT):
                nc.gpsimd.memset(augw[t][0:1, :], 1.0)

            # ---------------- init: weight transposes ----------------
            with tc.tile_pool(name=R+"initp", bufs=1) as initp:
                wtmp = initp.tile([P, 2, D_IN], F32, tag="wtmp")
                nc.sync.dma_start(wtmp[:], w_in[:].rearrange("(c p) d -> p c d", p=P))
                for ci in range(4):
                    for m in range(2):
                        tp = paux.tile([P, P], F32, tag="tp")
                        nc.tensor.transpose(tp[:], wtmp[:, m, ci * P:(ci + 1) * P], ident[:])
                        nc.scalar.activation(w_in_T[:, ci, m * P:(m + 1) * P], tp[:], Act.Copy)
                wtmp2 = initp.tile([P, 4, D_CB], F32, tag="wtmp")
                nc.sync.dma_start(wtmp2[:], w_out[:].rearrange("(c p) d -> p c d", p=P))
                for ci in range(4):
                    for m in range(2):
                        tp = paux.tile([P, P], F32, tag="tp")
                        nc.tensor.transpose(tp[:], wtmp2[:, ci, m * P:(m + 1) * P], ident[:])
                        nc.scalar.activation(w_out_T[:, m, ci * P:(ci + 1) * P], tp[:], Act.Copy)

                # ---------------- init: x -> r0_T, x_sq ----------------
                for b in range(8):  # 512-t blocks
                    xblk = initp.tile([P, 4, D_IN], F32, tag="xblk")
                    nc.sync.dma_start(
                        xblk[:], x[b * 512:(b + 1) * 512, :].rearrange("(c p) d -> p c d", p=P))
                    xT = initp.tile([P, 4, 512], F32, tag="xT")  # [din_p, din_c, t_in_blk]
                    for tb in range(4):
                        for db in range(4):
                            tp = paux.tile([P, P], F32, tag="tp")
                            nc.tensor.transpose(tp[:], xblk[:, tb, db * P:(db + 1) * P], ident[:])
                            nc.scalar.activation(xT[:, db, tb * P:(tb + 1) * P], tp[:], Act.Copy)
                    # r0_T chunks
                    for m in range(2):
                        pr = pdist.tile([P, 512], F32, tag="pd")
                        for ci in range(4):
                            nc.tensor.matmul(pr[:], w_in_T[:, ci, m * P:(m + 1) * P],
                                             xT[:, ci, :], start=(ci == 0), stop=(ci == 3))
                        for tb in range(4):
                            nc.scalar.activation(rT[m][b * 4 + tb][:],
                                                 pr[:, tb * P:(tb + 1) * P], Act.Copy)
                    # r0 natural per t-subtile -> x_sq
                    for tb in range(4):
                        t = b * 4 + tb
                        pn = paux.tile([P, D_CB], F32, tag="tp")
                        for ci in range(4):
                            nc.tensor.matmul(pn[:], xT[:, ci, tb * P:(tb + 1) * P],
                                             w_in_T[:, ci, :], start=(ci == 0), stop=(ci == 3))
                        sq = initp.tile([P, D_CB], F32, tag="sq")
                        nc.scalar.activation(sq[:], pn[:], Act.Square)
                        xs = smalls.tile([P, 1], F32, tag="xs")
                        nc.vector.tensor_reduce(xs[:], sq[:], axis=mybir.AxisListType.X,
                                                op=Alu.add)
                        nc.scalar.activation(nxsq[0][t][:], xs[:], Act.Copy, scale=-1.0)
                    for m in range(2):
                        for tb in range(4):
                            nc.vector.memzero(outT[m][b * 4 + tb][:])

            # ---------------- main: 8 codebook layers ----------------
            for q in range(Q):
                # layer prep: e2T (transposed, x2), etilde row (bf16)
                estage = lscratch.tile([P, 16, D_CB], F32, tag="estage",
                                             name=R+f"estage{q}")
                nc.sync.dma_start(
                    estage[:], emb[q * K:(q + 1) * K, :].rearrange("(c p) d -> p c d", p=P))
                e2T = [layer.tile([P, K], F32, tag=f"e2T{m}", name=R+f"e2T{m}_{q}")
                       for m in range(2)]
                for c in range(16):
                    for m in range(2):
                        tp = paux.tile([P, P], F32, tag="tp")
                        nc.tensor.transpose(tp[:], estage[:, c, m * P:(m + 1) * P], ident[:])
                        nc.scalar.activation(e2T[m][:, c * P:(c + 1) * P], tp[:], Act.Copy,
                                             scale=2.0)
                esq = smalls.tile([P, 16], F32, tag="esq")
                for c in range(16):
                    sqc = lscratch.tile([P, D_CB], F32, tag="sqc", name=R+f"sqc{q}_{c}")
                    nc.scalar.activation(sqc[:], estage[:, c, :], Act.Square)
                    nc.vector.tensor_reduce(esq[:, c:c + 1], sqc[:],
                                            axis=mybir.AxisListType.X, op=Alu.add)
                tpe = paux.tile([16, P], F32, tag="tp")
                nc.tensor.transpose(tpe[:], esq[:], ident[:])
                # grid-round e_sq to 2^-17 and 2^-16 (RNE via +2^23 magic), negate.
                # Row 0 of eneg: -etilde17 (applied to every row); row 1:
                # -(etilde16 - etilde17), applied only where x_sq >= 128 (the
                # [128,256) binade rounds dist at 2^-16).
                g17inv, g17 = float(2.0 ** 17), float(2.0 ** -17)
                g16inv, g16 = float(2.0 ** 16), float(2.0 ** -16)
                q1 = smalls.tile([16, P], F32, tag="q1")
                nc.scalar.activation(q1[:], tpe[:], Act.Identity, scale=g17inv,
                                     bias=bias_magic[:16, :])
                q2 = smalls.tile([16, P], F32, tag="q2")
                nc.scalar.activation(q2[:], q1[:], Act.Identity, scale=-g17,
                                     bias=bias_64[:16, :])
                q1b = smalls.tile([16, P], F32, tag="q1b")
                nc.scalar.activation(q1b[:], tpe[:], Act.Identity, scale=g16inv,
                                     bias=bias_magic[:16, :])
                q2c = smalls.tile([16, P], F32, tag="q2c")
                nc.scalar.activation(q2c[:], q1b[:], Act.Identity, scale=-g16,
                                     bias=bias_128[:16, :])
                qv = smalls.tile([16, P], F32, tag="qv")
                nc.vector.tensor_tensor(qv[:], q2c[:], q2[:], op=Alu.subtract)
                q2b = smalls.tile([16, P], BF16, tag="q2b")
                nc.vector.tensor_copy(q2b[:], q2[:])
                qvb = smalls.tile([16, P], BF16, tag="qvb")
                nc.vector.tensor_copy(qvb[:], qv[:])
                eneg = layer.tile([2, K], BF16, tag="eneg")
                nc.sync.dma_start(eneg[0:1, :], q2b[:])
                nc.sync.dma_start(eneg[1:2, :], qvb[:])

                cur, nxt = nxsq[q % 2], nxsq[(q + 1) % 2]
                for t in range(NT):
                    bflag = smalls.tile([P, 1], F32, tag="bflag")
                    nc.vector.tensor_single_scalar(bflag[:], cur[t][:], -128.0,
                                                   Alu.is_le)
                    pbf = paux.tile([1, P], F32, tag="tp")
                    nc.tensor.transpose(pbf[:], bflag[:], ident[:])
                    bsb = smalls.tile([1, P], BF16, tag="bsb")
                    nc.scalar.activation(bsb[:], pbf[:], Act.Copy)
                    nc.sync.dma_start(augw[t][1:2, :], bsb[:])
                    pd = [pdist.tile([P, 512], F32, tag="pd", name=R+f"pd{q}_{t}_{ch}")
                          for ch in range(4)]
                    for pair in ((0, 1), (2, 3)):
                        for m in range(2):
                            for ch in pair:
                                nc.tensor.matmul(pd[ch][:], rT[m][t][:],
                                                 e2T[m][:, ch * 512:(ch + 1) * 512],
                                                 start=(m == 0), stop=False)
                        for ch in pair:
                            nc.tensor.matmul(pd[ch][:], augw[t][:],
                                             eneg[:, ch * 512:(ch + 1) * 512],
                                             start=False, stop=True)
                    s1 = work.tile([P, K], F32, tag="s1")
                    for ch in range(4):
                        nc.scalar.activation(s1[:, ch * 512:(ch + 1) * 512], pd[ch][:],
                                             Act.Identity, bias=cur[t][:], scale=1.0)
                    m8 = smalls.tile([P, 8], F32, tag="m8")
                    nc.vector.max(m8[:], s1[:])
                    idx = smalls.tile([P, 8], U32, tag="idx")
                    nc.vector.max_index(idx[:], m8[:], s1[:])
                    # next layer bias = max(s1) = -dist_min = -x_sq_next
                    nc.scalar.activation(nxt[t][:], m8[:, 0:1], Act.Copy)
                    idxg = smalls.tile([P, 1], U32, tag="idxg")
                    nc.vector.tensor_single_scalar(idxg[:], idx[:, 0:1], float(q * K), Alu.add)
                    qrow = smalls.tile([P, D_CB], F32, tag="qrow")
                    if NO_GATHER:
                        nc.sync.dma_start(qrow[:], emb[q * K:q * K + P, :])
                    else:
                        nc.gpsimd.indirect_dma_start(
                            out=qrow[:], out_offset=None, in_=emb[:, :],
                            in_offset=bass.IndirectOffsetOnAxis(ap=idxg[:, 0:1], axis=0))
                    ptq = pqt.tile([P, D_CB], F32, tag="ptq")
                    for m in range(2):
                        nc.tensor.transpose(ptq[:, m * P:(m + 1) * P],
                                            qrow[:, m * P:(m + 1) * P], ident[:])
                    for m in range(2):
                        nc.vector.tensor_tensor(rT[m][t][:], rT[m][t][:],
                                                ptq[:, m * P:(m + 1) * P], op=Alu.subtract)
                        nc.vector.tensor_tensor(outT[m][t][:], outT[m][t][:],
                                                ptq[:, m * P:(m + 1) * P], op=Alu.add)

            # ---------------- output projection ----------------
            for t in range(NT):
                py = pdist.tile([P, D_IN], F32, tag="pd")
                for m in range(2):
                    nc.tensor.matmul(py[:], outT[m][t][:], w_out_T[:, m, :],
                                     start=(m == 0), stop=(m == 1))
                ysb = work.tile([P, D_IN], F32, tag="ysb")
                nc.scalar.activation(ysb[:], py[:], Act.Copy)
                nc.sync.dma_start(y[t * P:(t + 1) * P, :], ysb[:])

    nc.compile()
    return nc


_NC_CACHE = None


def _get_nc(reps=1):
    global _NC_CACHE
    if _NC_CACHE is None:
        _NC_CACHE = _build(reps)
    return _NC_CACHE


def kernel(x_td, w_in, w_out, embeddings, _trace=False):
    x_td = np.ascontiguousarray(np.asarray(x_td, dtype=np.float32))
    w_in = np.ascontiguousarray(np.asarray(w_in, dtype=np.float32))
    w_out = np.ascontiguousarray(np.asarray(w_out, dtype=np.float32))
    emb2d = np.ascontiguousarray(
        np.asarray(embeddings, dtype=np.float32).reshape(Q * K, D_CB))

    nc = _get_nc()
    in_maps = [
        {"x": x_td[i * T_LOC:(i + 1) * T_LOC], "w_in": w_in, "w_out": w_out,
         "emb": emb2d}
        for i in range(N_CORES)
    ]
    res = run_bass_kernel_spmd(nc, in_maps, core_ids=list(range(N_CORES)),
                               trace=_trace)
    out = np.concatenate([r["y"] for r in res.results], axis=0)
    if _trace:
        kernel.last_exec_time_ns = res.exec_time_ns
        kernel.last_results = res
    return out


if __name__ == "__main__":
    rng = np.random.default_rng(0)
    xs = rng.standard_normal((T, D_IN)).astype(np.float32)
    wi = rng.uniform(-1, 1, (D_CB, D_IN)).astype(np.float32) / np.sqrt(D_IN)
    wo = rng.uniform(-1, 1, (D_IN, D_CB)).astype(np.float32) / np.sqrt(D_CB)
    em = (rng.uniform(-1, 1, (Q, K, D_CB)).astype(np.float32) / K)
    out = kernel(xs, wi, wo, em)
    print("kernel ran, out", out.shape, out.dtype, float(np.abs(out).max()))

